# revision 31
# baseline (speedup 1.0000x reference)
"""DIEN layer (GRU + attention + AUGRU) Trainium2 Bass kernel.

Pure data parallel across 8 NeuronCores: batch 2048 -> 256 per core.

Device layout: features on SBUF partitions, batch on the free dim; all
matmuls keep state in [feat, batch] layout so the recurrence never
transposes.  Ragged sequences: for t >= seq_len(b) the update gate is
saturated (v = 1-u -> 0) by adding -BIG to the (negated) u-gate
preactivation via a K=1 matmul, which freezes h exactly; the attention
softmax masks dead positions to exp(NEG-max) = 0 so alphas are exactly
0 there and the AUGRU also freezes.  Compute dtype for matmuls and
elementwise is bf16 (fp32 PSUM accumulation, fp32 softmax); set
dtc_name="f32" for a full-precision (4x slower matmul) variant.
"""

import sys

sys.path.insert(0, "/opt/trn_rl_repo")

import numpy as np
import ml_dtypes

import concourse.bacc as bacc
import concourse.mybir as mybir
import concourse.tile as tile
from concourse.bass_utils import run_bass_kernel_spmd

B, T, D, H = 2048, 200, 128, 128
NCORES = 8
BL = B // NCORES

BIG = 30000.0
NEG = np.float32(-(2.0**32) + 1.0)

F32 = mybir.dt.float32
BF16 = mybir.dt.bfloat16


def build_program(T_=T, BL_=BL, dtc_name="bf16", parts="all", windows=None):
    """Build the single-core program (run SPMD across 8 cores)."""
    nc = bacc.Bacc("TRN2", target_bir_lowering=False, debug=False)
    dt = F32
    dtc = BF16 if dtc_name == "bf16" else F32
    wins = list(windows) if windows is not None else [BL_] * T_
    assert len(wins) == T_ and all(0 <= w <= BL_ for w in wins)

    def dram(name, shape, dty=dt, kind="ExternalInput"):
        return nc.dram_tensor(name, shape, dty, kind=kind).ap()

    # ---- external inputs (per core); compute-dtype tensors use dtc ----
    xT = dram("xT", [T_, D, BL_], dtc)
    qT = dram("qT", [D, BL_], dtc)
    qN = dram("qN", [BL_, D])
    hsum = dram("hsum", [BL_, D])
    validBT = dram("validBT", [BL_, T_], mybir.dt.uint8)
    negmBT = dram("negmBT", [BL_, T_])
    deadT = dram("deadT", [T_, BL_], dtc)

    w1x = dram("w1x", [D, 2 * H], dtc)
    w1h = dram("w1h", [H, 2 * H], dtc)
    w1hn = dram("w1hn", [H, 2 * H], dtc)
    c1x = dram("c1x", [D, H], dtc)
    c1h = dram("c1h", [H, H], dtc)
    gb1n = dram("gb1n", [2 * H])
    gbw1r = dram("gbw1r", [1, 128], dtc)
    gbw1u = dram("gbw1u", [1, 128], dtc)
    gbw2r = dram("gbw2r", [1, 128], dtc)
    gbw2u = dram("gbw2u", [1, 128], dtc)
    cb1 = dram("cb1", [H])
    w2x = dram("w2x", [H, 2 * H], dtc)
    w2h = dram("w2h", [H, 2 * H], dtc)
    w2hn = dram("w2hn", [H, 2 * H], dtc)
    c2x = dram("c2x", [H, H], dtc)
    c2h = dram("c2h", [H, H], dtc)
    gb2n = dram("gb2n", [2 * H])
    cb2 = dram("cb2", [H])

    wq = dram("wq", [D, H], dtc)
    bq = dram("bq", [H])
    pra = dram("pra", [H])
    pra1m = dram("pra1m", [H])
    w1apc = dram("w1apc", [H, 80], dtc)
    w1bmc = dram("w1bmc", [H, 80], dtc)
    w1d = dram("w1d", [H, 80], dtc)
    b1 = dram("b1", [80])
    w2a = dram("w2a", [80, 40], dtc)
    b2 = dram("b2", [40])
    w3 = dram("w3", [40, 1], dtc)
    ident = dram("ident", [128, 128])
    identc = dram("identc", [128, 128], dtc)
    negbig = dram("negbig", [1, 128], dtc)
    onescol = dram("onescol", [1, 128], dtc)

    out = dram("out", [BL_, 3 * D + H], dt, kind="ExternalOutput")

    # DRAM scratch: (1 - alpha) rows, compute dtype
    aTd = nc.dram_tensor("aTd", [T_, BL_], dtc).ap()

    DCH = 8  # steps of deadrow/alpha rows per [1, DCH*BL] chunk

    SIG = mybir.ActivationFunctionType.Sigmoid
    TANH = mybir.ActivationFunctionType.Tanh
    EXP = mybir.ActivationFunctionType.Exp
    RELU = mybir.ActivationFunctionType.Relu
    COPYF = mybir.ActivationFunctionType.Copy
    AX = mybir.AxisListType.X
    MUL = mybir.AluOpType.mult
    SUB = mybir.AluOpType.subtract
    ADDOP = mybir.AluOpType.add
    MAXOP = mybir.AluOpType.max

    with tile.TileContext(nc) as tc:
        with tc.tile_pool(name="wts", bufs=1) as wp:

            def load_w(ap, shape, tag, col=False, dty=dtc):
                t_ = wp.tile(shape, dty, tag=tag, name=tag)
                if col:
                    n = ap.shape[0]
                    if n <= 128:
                        nc.sync.dma_start(t_[:, 0:1], ap.rearrange("(h a) -> h a", a=1))
                    else:
                        nc.sync.dma_start(t_[:], ap.rearrange("(a h) -> h a", h=128))
                else:
                    nc.sync.dma_start(t_[:], ap)
                return t_

            W1x = load_w(w1x, [D, 2 * H], "W1x")
            W1h = load_w(w1h, [H, 2 * H], "W1h")
            W1hn = load_w(w1hn, [H, 2 * H], "W1hn")
            C1x = load_w(c1x, [D, H], "C1x")
            C1h = load_w(c1h, [H, H], "C1h")
            W2x = load_w(w2x, [H, 2 * H], "W2x")
            W2h = load_w(w2h, [H, 2 * H], "W2h")
            W2hn = load_w(w2hn, [H, 2 * H], "W2hn")
            C2x = load_w(c2x, [H, H], "C2x")
            C2h = load_w(c2h, [H, H], "C2h")
            GB1 = load_w(gb1n, [128, 2], "GB1", col=True, dty=dt)
            GBW1 = (load_w(gbw1r, [1, 128], "GBW1r"),
                    load_w(gbw1u, [1, 128], "GBW1u"))
            GBW2 = (load_w(gbw2r, [1, 128], "GBW2r"),
                    load_w(gbw2u, [1, 128], "GBW2u"))
            CB1 = load_w(cb1, [H, 1], "CB1", col=True, dty=dt)
            GB2 = load_w(gb2n, [128, 2], "GB2", col=True, dty=dt)
            CB2 = load_w(cb2, [H, 1], "CB2", col=True, dty=dt)
            WQ = load_w(wq, [D, H], "WQ")
            BQ = load_w(bq, [H, 1], "BQ", col=True, dty=dt)
            PRA = load_w(pra, [H, 1], "PRA", col=True, dty=dt)
            PRA1M = load_w(pra1m, [H, 1], "PRA1M", col=True, dty=dt)
            W1APC = load_w(w1apc, [H, 80], "W1APC")
            W1BMC = load_w(w1bmc, [H, 80], "W1BMC")
            W1D = load_w(w1d, [H, 80], "W1D")
            B1 = load_w(b1, [80, 1], "B1", col=True, dty=dt)
            W2A = load_w(w2a, [80, 40], "W2A")
            B2 = load_w(b2, [40, 1], "B2", col=True, dty=dt)
            W3 = load_w(w3, [40, 1], "W3")
            IDN = load_w(ident, [128, 128], "IDN", dty=dt)
            IDNC = load_w(identc, [128, 128], "IDNC")
            NBIG = load_w(negbig, [1, 128], "NBIG")
            ONEC = load_w(onescol, [1, 128], "ONEC")

            QT = wp.tile([D, BL_], dtc, tag="QT", name="QT")
            nc.sync.dma_start(QT[:], qT)
            # rnn1 lives in SBUF: [H, T*BL], one BL-wide slab per step
            RNN1 = wp.tile([H, T_ * BL_], dtc, tag="RNN1", name="RNN1")
            HZERO = wp.tile([H, BL_], dtc, tag="HZERO", name="HZERO")
            nc.vector.memset(HZERO[:], 0.0)
            nbh = (BL_ + 127) // 128
            VAL, NEGM = [], []
            for i in range(nbh):
                p = min(128, BL_ - i * 128)
                v_ = wp.tile([128, T_], mybir.dt.uint8, tag=f"VAL{i}", name=f"VAL{i}")
                nc.sync.dma_start(v_[0:p, :], validBT[i * 128 : i * 128 + p, :])
                VAL.append(v_)
                n_ = wp.tile([128, T_], dt, tag=f"NEGM{i}", name=f"NEGM{i}")
                nc.sync.dma_start(n_[0:p, :], negmBT[i * 128 : i * 128 + p, :])
                NEGM.append(n_)

            # ---- attention query path (once) ----
            with (
                tc.tile_pool(name="ps_small", bufs=2, space="PSUM") as psp,
                tc.tile_pool(name="setup_tmp", bufs=2) as stp,
            ):
                p_qp = psp.tile([H, BL_], dt, tag="p_qp")
                nc.tensor.matmul(p_qp[:], WQ[:], QT[:], start=True, stop=True)
                qpre = stp.tile([H, BL_], dt, tag="qpre")
                nc.scalar.add(qpre[:], p_qp[:], BQ[:, 0:1])
                r1 = stp.tile([H, BL_], dt, tag="r1")
                nc.scalar.activation(r1[:], qpre[:], RELU, scale=PRA1M[:, 0:1])
                QP = wp.tile([H, BL_], dtc, tag="QP", name="QP")
                nc.vector.scalar_tensor_tensor(
                    QP[:], qpre[:], PRA[:, 0:1], r1[:], op0=MUL, op1=ADDOP
                )
                p_qc = psp.tile([80, BL_], dt, tag="p_qc")
                nc.tensor.matmul(p_qc[:], W1APC[:], QP[:], start=True, stop=True)
                QC = wp.tile([80, BL_], dt, tag="QC", name="QC")
                nc.scalar.add(QC[:], p_qc[:], B1[:, 0:1])

            # =================== GRU recurrences ===================
            def gru_pass(W_x, W_h, W_hn, C_x, C_h, GBn, CBc, x_of_t, store_rnn1,
                         use_alpha, hout=None, post_step=None, post_g0=None):
                # store_rnn1: write h into RNN1 SBUF regions (no h tiles)
                # Recurrent gate matmuls consume (vc, p_) of the previous step
                # (h' = vc - p_) so the h'-subtract runs off the critical path.
                with (
                    tc.tile_pool(name="g_x", bufs=4) as xp,
                    tc.tile_pool(name="g_h", bufs=3) as hp,
                    tc.tile_pool(name="g_rv", bufs=3) as rvp,
                    tc.tile_pool(name="g_tmp", bufs=6) as tp,
                    tc.tile_pool(name="g_dead", bufs=3) as dp,
                    tc.tile_pool(name="g_pg", bufs=2, space="PSUM") as pgp,
                    tc.tile_pool(name="g_pc", bufs=1, space="PSUM") as pcp,
                    tc.tile_pool(name="g_pa", bufs=1, space="PSUM") as pap,
                ):
                    NG = 2 if BL_ % 2 == 0 else 1  # pipeline groups (batch halves)
                    GW = BL_ // NG
                    inplace_h = use_alpha  # AUGRU: frozen suffix is the output
                    hs_ = []
                    for g in range(NG):
                        if store_rnn1:
                            hs_.append(HZERO[:, g * GW : (g + 1) * GW])
                        else:
                            h_g = hp.tile([H, GW], dtc, tag=f"h{g}", name=f"h{g}")
                            nc.vector.memset(h_g[:], 0.0)
                            hs_.append(h_g)
                    dead_ch = None
                    alpha_ch = None
                    prev_vc = [None] * NG
                    prev_p = [None] * NG
                    for t in range(T_):
                        j = t % DCH
                        if j == 0:
                            n_in = min(DCH, T_ - t)
                            dead_ch = dp.tile([1, DCH * BL_], dtc, tag="dead")
                            nc.sync.dma_start(
                                dead_ch[0:1, 0 : n_in * BL_],
                                deadT[t : t + n_in, :].rearrange(
                                    "(c a) b -> c (a b)", c=1),
                            )
                            if use_alpha:
                                alpha_ch = dp.tile([1, DCH * BL_], dtc, tag="alpha")
                                nc.sync.dma_start(
                                    alpha_ch[0:1, 0 : n_in * BL_],
                                    aTd[t : t + n_in, :].rearrange(
                                        "(c a) b -> c (a b)", c=1),
                                )
                        if wins[t] == 0:
                            continue
                        x_t = x_of_t(xp, t)
                        acts = []
                        for g in range(NG):
                            wg = min(max(wins[t] - g * GW, 0), GW)
                            if wg > 0:
                                acts.append((g, wg, g * GW))
                        # Group-major emission; recurrent matmuls consume
                        # (vc, p_) of the previous step so the h'-subtract is
                        # off the matmul critical path.
                        for g, wg, lo in acts:
                            xg = x_t[:, lo : lo + wg]
                            hg = hs_[g][:, 0:wg]
                            pv, pp = prev_vc[g], prev_p[g]
                            pg = pgp.tile([128, 2 * GW], dt, tag=f"pg{g}",
                                          name=f"pg{g}")
                            nc.tensor.matmul(pg[:, 0:wg], W_x[:, 0:H], xg,
                                             start=True, stop=(pv is None))
                            if pv is not None:
                                nc.tensor.matmul(pg[:, 0:wg], W_hn[:, 0:H],
                                                 pp[:, 0:wg],
                                                 start=False, stop=False)
                                nc.tensor.matmul(pg[:, 0:wg], W_h[:, 0:H],
                                                 pv[:, 0:wg],
                                                 start=False, stop=True)
                            nc.tensor.matmul(pg[:, GW : GW + wg], W_x[:, H:], xg,
                                             start=True, stop=False)
                            nc.tensor.matmul(
                                pg[:, GW : GW + wg], NBIG[:],
                                dead_ch[0:1, j * BL_ + lo : j * BL_ + lo + wg],
                                start=False, stop=(pv is None))
                            if pv is not None:
                                nc.tensor.matmul(pg[:, GW : GW + wg],
                                                 W_hn[:, H:], pp[:, 0:wg],
                                                 start=False, stop=False)
                                nc.tensor.matmul(pg[:, GW : GW + wg],
                                                 W_h[:, H:], pv[:, 0:wg],
                                                 start=False, stop=True)
                            rv = rvp.tile([128, 2 * GW], dtc, tag=f"rv{g}",
                                          name=f"rv{g}")
                            nc.scalar.activation(rv[:, 0:wg], pg[:, 0:wg], SIG,
                                                 bias=GBn[:, 0:1])
                            nc.scalar.activation(rv[:, GW : GW + wg],
                                                 pg[:, GW : GW + wg], SIG,
                                                 bias=GBn[:, 1:2])
                            rh = tp.tile([H, GW], dtc, tag=f"rh{g}",
                                         name=f"rh{g}")
                            nc.vector.tensor_mul(rh[:, 0:wg], rv[:, 0:wg], hg)
                            pc = pcp.tile([H, GW], dt, tag=f"pc{g}",
                                          name=f"pc{g}")
                            nc.tensor.matmul(pc[:, 0:wg], C_x[:], xg,
                                             start=True, stop=False)
                            nc.tensor.matmul(pc[:, 0:wg], C_h[:], rh[:, 0:wg],
                                             start=False, stop=True)
                            c = tp.tile([H, GW], dtc, tag=f"c{g}", name=f"c{g}")
                            nc.scalar.activation(c[:, 0:wg], pc[:, 0:wg], TANH,
                                                 bias=CBc[:, 0:1])
                            if use_alpha:
                                pa = pap.tile([128, GW], dt, tag=f"pa{g}",
                                              name=f"pa{g}")
                                nc.tensor.matmul(
                                    pa[:, 0:wg], ONEC[:],
                                    alpha_ch[0:1, j * BL_ + lo : j * BL_ + lo + wg],
                                    start=True, stop=True)
                                u1 = tp.tile([H, GW], dtc, tag=f"u1{g}",
                                             name=f"u1{g}")
                                nc.vector.scalar_tensor_tensor(
                                    u1[:, 0:wg], rv[:, GW : GW + wg], 1.0,
                                    pa[:, 0:wg], op0=SUB, op1=MUL)
                                p_ = tp.tile([H, GW], dtc, tag=f"p_{g}",
                                             name=f"p_{g}")
                                nc.vector.tensor_mul(p_[:, 0:wg], u1[:, 0:wg], hg)
                                vc = tp.tile([H, GW], dtc, tag=f"vc{g}",
                                             name=f"vc{g}")
                                nc.vector.scalar_tensor_tensor(
                                    vc[:, 0:wg], u1[:, 0:wg], 1.0, c[:, 0:wg],
                                    op0=ADDOP, op1=MUL)
                            else:
                                p_ = tp.tile([H, GW], dtc, tag=f"p_{g}",
                                             name=f"p_{g}")
                                nc.vector.scalar_tensor_tensor(
                                    p_[:, 0:wg], rv[:, GW : GW + wg], 1.0, hg,
                                    op0=SUB, op1=MUL)
                                vc = tp.tile([H, GW], dtc, tag=f"vc{g}",
                                             name=f"vc{g}")
                                nc.vector.tensor_mul(vc[:, 0:wg],
                                                     rv[:, GW : GW + wg],
                                                     c[:, 0:wg])
                            if inplace_h:
                                nc.vector.tensor_sub(hs_[g][:, 0:wg],
                                                     vc[:, 0:wg], p_[:, 0:wg])
                            elif store_rnn1:
                                reg = RNN1[:, t * BL_ + lo : t * BL_ + lo + GW]
                                nc.vector.tensor_sub(reg[:, 0:wg], vc[:, 0:wg],
                                                     p_[:, 0:wg])
                                hs_[g] = reg
                            else:
                                h2 = hp.tile([H, GW], dtc, tag=f"h{g}",
                                             name=f"h{g}")
                                nc.vector.tensor_sub(h2[:, 0:wg], vc[:, 0:wg],
                                                     p_[:, 0:wg])
                                hs_[g] = h2
                            prev_vc[g], prev_p[g] = vc, p_
                            if post_g0 is not None and g == 0:
                                post_g0(t)
                        if post_step is not None:
                            post_step(t, hs_, GW)
                    if hout is not None:
                        for g in range(NG):
                            nc.vector.tensor_copy(
                                hout[:, g * GW : (g + 1) * GW], hs_[g][:])

            XB = 4
            xT_p = xT.rearrange("t p b -> p t b")
            xcache = {}

            def x_from_xT(xp, t):
                t0 = (t // XB) * XB
                if t0 not in xcache:
                    nb = min(XB, T_ - t0)
                    xc = xp.tile([D, XB * BL_], dtc, tag="x", name=f"x{t0}")
                    nc.sync.dma_start(
                        xc[:].rearrange("p (a b) -> p a b", a=XB)[:, 0:nb, :],
                        xT_p[:, t0 : t0 + nb, :])
                    xcache.clear()
                    xcache[t0] = xc
                return xcache[t0][:, (t - t0) * BL_ : (t - t0 + 1) * BL_]

            hfin = wp.tile([H, BL_], dtc, tag="hfin", name="hfin")
            nc.vector.memset(hfin[:], 0.0)

            # ============ GRU1 with interleaved per-step attention ============
            do_attn = "attn" in parts or parts == "all"
            with (
                tc.tile_pool(name="a_tmp", bufs=4) as atp,
                tc.tile_pool(name="a_ps", bufs=2, space="PSUM") as app,
            ):
                # packed psum bank per step: s1 [0:BL], s2 [BL:2BL]; the two
                # score columns reuse s1's cols [0:2] after a1 consumed them
                SC0, SC1, SC2 = 0, BL_, 0
                scores_sb = wp.tile([128, 2 * T_], dt, tag="scores",
                                    name="scores")
                nc.vector.memset(scores_sb[:], 0.0)
                sc_v = scores_sb[:].rearrange("p (i t) -> p i t", i=2)

                # Attention score FCN processed in PAIRS of GRU steps,
                # lag-2 pipelined into the ACT-idle slots of the recurrence:
                # psA MMs accumulate per step; sigma1 for a completed pair
                # fires at the next step's post_g0 slot; W2A/sigma2/W3/score
                # copies at the next post_step. Column layout per pair bank:
                # a1pre even step at [0:80, 0:BL], odd at [0:80, BL:2BL];
                # a2pre overwrites [0:40, ...] after sigma1; W3 scores land in
                # cols [0:4] as [i0e, i0o, i1e, i1o]. Stale-PSUM columns of
                # the combined ranges are finite garbage, masked in softmax.
                cur = {}
                rdy = {}
                fin = {}

                def attn_sig1(t):
                    if not do_attn or not rdy:
                        return
                    psA = rdy["psA"]
                    wtot = rdy["w0"] + rdy["w1"]
                    a1s = atp.tile([80, 2 * BL_], dtc, tag="a1s", name="a1s")
                    nc.scalar.activation(a1s[:, 0:wtot], psA[0:80, 0:wtot],
                                         SIG, bias=B1[:, 0:1])
                    rdy["a1s"] = a1s
                    fin.update(rdy)
                    rdy.clear()

                def attn_fin():
                    psA, a1s = fin["psA"], fin["a1s"]
                    t0, w0, t1, w1 = fin["t0"], fin["w0"], fin["t1"], fin["w1"]
                    wtot = w0 + w1
                    nc.tensor.matmul(psA[0:40, 0:wtot], W2A[:], a1s[:, 0:wtot],
                                     start=True, stop=True)
                    a2 = atp.tile([40, 2 * BL_], dtc, tag="a2", name="a2")
                    nc.scalar.activation(a2[:, 0:wtot], psA[0:40, 0:wtot],
                                         SIG, bias=B2[:, 0:1])
                    for i in range(nbh):
                        p = min(128, w0 - i * 128)
                        if p <= 0:
                            continue
                        p1 = max(0, min(128, w1 - i * 128))
                        nc.tensor.matmul(
                            psA[0:p, 2 * i : 2 * i + 1],
                            a2[:, i * 128 : i * 128 + p], W3[:],
                            start=True, stop=True)
                        if p1 > 0:
                            nc.tensor.matmul(
                                psA[0:p1, 2 * i + 1 : 2 * i + 2],
                                a2[:, w0 + i * 128 : w0 + i * 128 + p1],
                                W3[:], start=True, stop=True)
                            nc.vector.tensor_copy(
                                sc_v[:, i, :][0:p, t0 : t0 + 2],
                                psA[0:p, 2 * i : 2 * i + 2])
                        else:
                            nc.vector.tensor_copy(
                                sc_v[:, i, :][0:p, t0 : t0 + 1],
                                psA[0:p, 2 * i : 2 * i + 1])
                    fin.clear()

                def attn_step(t, hs_, GW):
                    if not do_attn:
                        return
                    if fin:
                        attn_fin()
                    if t is None:
                        return
                    w = wins[t]
                    if not cur:
                        psA = app.tile([128, 2 * BL_], dt, tag="psA",
                                       name="psA")
                        cur.update(t0=t, w0=w, psA=psA, off=0)
                        off = 0
                    else:
                        psA = cur["psA"]
                        off = cur["w0"]
                    for g in range(len(hs_)):
                        wg = min(max(w - g * GW, 0), GW)
                        if wg == 0:
                            continue
                        lo = g * GW
                        hg = hs_[g][:, 0:wg]
                        prod = atp.tile([H, GW], dtc, tag=f"aprod{g}",
                                        name=f"aprod{g}")
                        nc.vector.tensor_mul(prod[:, 0:wg], hg,
                                             QP[:, lo : lo + wg])
                        dst = psA[0:80, off + lo : off + lo + wg]
                        nc.tensor.matmul(dst, W1BMC[:], hg,
                                         start=True, stop=False)
                        nc.tensor.matmul(dst, W1APC[:], QP[:, lo : lo + wg],
                                         start=False, stop=False)
                        nc.tensor.matmul(dst, W1D[:], prod[:, 0:wg],
                                         start=False, stop=True)
                    if off > 0:
                        cur.update(t1=t, w1=w)
                        rdy.update(cur)
                        cur.clear()

                use_attn = "gru1" in parts or parts == "all"
                gru_pass(W1x, W1h, W1hn, C1x, C1h, GB1, CB1, x_from_xT,
                         True, False,
                         post_step=attn_step if use_attn else None,
                         post_g0=attn_sig1 if use_attn else None)
                if use_attn and do_attn:
                    if fin:
                        attn_fin()
                    if rdy:
                        attn_sig1(None)
                        attn_fin()

                # masked softmax; store abar = (1 - alpha) transposed to aTd
                nbh_sm = nbh if do_attn else 0
                with (
                    tc.tile_pool(name="a_sm", bufs=1) as smp,
                    tc.tile_pool(name="a_tr", bufs=2, space="PSUM") as trp,
                ):
                    for i in range(nbh_sm):
                        p = min(128, BL_ - i * 128)
                        sm = smp.tile([128, T_], dt, tag=f"sm{i}", name=f"sm{i}")
                        nc.vector.select(sm[0:p, :], VAL[i][0:p, :],
                                         sc_v[:, i, :][0:p, :], NEGM[i][0:p, :])
                        nmx = smp.tile([128, 1], dt, tag=f"nmx{i}", name=f"nmx{i}")
                        nc.vector.tensor_reduce(
                            nmx[0:p, :], sm[0:p, :], axis=AX, op=MAXOP, negate=True)
                        ex = smp.tile([128, T_], dt, tag=f"ex{i}", name=f"ex{i}")
                        nc.scalar.activation(ex[0:p, :], sm[0:p, :], EXP,
                                             bias=nmx[0:p, 0:1])
                        sume = smp.tile([128, 1], dt, tag=f"sume{i}", name=f"sume{i}")
                        nc.vector.tensor_reduce(
                            sume[0:p, :], ex[0:p, :], axis=AX, op=ADDOP)
                        rec = smp.tile([128, 1], dt, tag=f"rec{i}", name=f"rec{i}")
                        nc.vector.reciprocal(rec[0:p, :], sume[0:p, :])
                        alp = smp.tile([128, T_], dt, tag=f"alp{i}", name=f"alp{i}")
                        nc.vector.tensor_scalar_mul(alp[0:p, :], ex[0:p, :],
                                                    rec[0:p, 0:1])
                        for c0 in range(0, T_, 128):
                            w2_ = min(128, T_ - c0)
                            pt = trp.tile([128, 128], dt, tag="p_tr")
                            nc.tensor.transpose(
                                pt[0:w2_, 0:p], alp[0:p, c0 : c0 + w2_],
                                IDN[0:p, 0:p])
                            st = smp.tile([128, 128], dtc, tag="st", name="st")
                            # abar = 1 - alpha, cast to compute dtype
                            nc.scalar.activation(
                                st[0:w2_, 0:p], pt[0:w2_, 0:p], COPYF,
                                bias=1.0, scale=-1.0)
                            nc.sync.dma_start(
                                aTd[c0 : c0 + w2_, i * 128 : i * 128 + p],
                                st[0:w2_, 0:p])

            # =================== AUGRU ===================
            def x_from_rnn1(xp, t):
                return RNN1[:, t * BL_ : (t + 1) * BL_]

            if "augru" in parts or parts == "all":
                gru_pass(W2x, W2h, W2hn, C2x, C2h, GB2, CB2, x_from_rnn1,
                         False, True, hout=hfin)

            # =================== output assembly ===================
            with (
                tc.tile_pool(name="o_t", bufs=2) as otp,
                tc.tile_pool(name="o_p", bufs=2, space="PSUM") as opp,
            ):
                for i in range(nbh if parts == "all" else 0):
                    p = min(128, BL_ - i * 128)
                    qs = otp.tile([128, D], dt, tag="qs")
                    nc.sync.dma_start(qs[0:p, :], qN[i * 128 : i * 128 + p, :])
                    hs = otp.tile([128, D], dt, tag="hs")
                    nc.sync.dma_start(hs[0:p, :], hsum[i * 128 : i * 128 + p, :])
                    pr = otp.tile([128, D], dt, tag="pr")
                    nc.vector.tensor_mul(pr[0:p, :], qs[0:p, :], hs[0:p, :])
                    nc.sync.dma_start(out[i * 128 : i * 128 + p, 0:D], qs[0:p, :])
                    nc.sync.dma_start(out[i * 128 : i * 128 + p, D : 2 * D], hs[0:p, :])
                    nc.sync.dma_start(out[i * 128 : i * 128 + p, 2 * D : 3 * D],
                                      pr[0:p, :])
                    ptr = opp.tile([128, 128], dtc, tag="ptr")
                    nc.tensor.transpose(ptr[0:p, :], hfin[:, i * 128 : i * 128 + p],
                                        IDNC[:])
                    ht = otp.tile([128, H], dt, tag="ht")
                    nc.scalar.copy(ht[0:p, :], ptr[0:p, :])
                    nc.sync.dma_start(out[i * 128 : i * 128 + p, 3 * D :], ht[0:p, :])

    nc.compile()
    return nc


def host_prep(item_eb, item_his_eb, item_his_eb_sum, mask,
              gk1, gb1, ck1, cb1,
              wq, bq, prelu_alpha, w1, b1, w2, b2, w3, b3,
              gk2, gb2, ck2, cb2, T_=T, BL_=BL, ncores=NCORES, dtc_name="bf16"):
    """Shard + preprocess. Samples are sorted by seq_len (descending) and
    dealt round-robin to cores so every core shares one length profile;
    returns (in_maps, perm, windows) where windows[t] is the 32-rounded
    max active-batch width at step t (same on every core by construction),
    and out[perm] = concat(core outputs) restores the original order."""
    f = np.float32
    fc = ml_dtypes.bfloat16 if dtc_name == "bf16" else np.float32

    w1x = np.ascontiguousarray(gk1[:D]).astype(f)
    w1h = np.ascontiguousarray(gk1[D:]).astype(f)
    w1x[:, H:] = -w1x[:, H:]
    w1h[:, H:] = -w1h[:, H:]
    gb1n = np.asarray(gb1, f).copy()
    gb1n[H:] = -gb1n[H:]
    w2x_ = np.ascontiguousarray(gk2[:H]).astype(f)
    w2h_ = np.ascontiguousarray(gk2[H:]).astype(f)
    w2x_[:, H:] = -w2x_[:, H:]
    w2h_[:, H:] = -w2h_[:, H:]
    gb2n = np.asarray(gb2, f).copy()
    gb2n[H:] = -gb2n[H:]
    shared = dict(
        w1x=w1x.astype(fc), w1h=w1h.astype(fc), w1hn=(-w1h).astype(fc),
        w2hn=(-w2h_).astype(fc),
        c1x=np.ascontiguousarray(ck1[:D]).astype(fc),
        c1h=np.ascontiguousarray(ck1[D:]).astype(fc),
        gb1n=gb1n, cb1=np.asarray(cb1, f),
        w2x=w2x_.astype(fc), w2h=w2h_.astype(fc),
        c2x=np.ascontiguousarray(ck2[:H]).astype(fc),
        c2h=np.ascontiguousarray(ck2[H:]).astype(fc),
        gb2n=gb2n, cb2=np.asarray(cb2, f),
        gbw1r=gb1n[None, :H].astype(fc), gbw1u=gb1n[None, H:].astype(fc),
        gbw2r=gb2n[None, :H].astype(fc), gbw2u=gb2n[None, H:].astype(fc),
        wq=np.asarray(wq).astype(fc), bq=np.asarray(bq, f),
        pra=np.asarray(prelu_alpha, f),
        pra1m=(1.0 - np.asarray(prelu_alpha, f)),
        w1apc=np.ascontiguousarray(w1[:H] + w1[2 * H : 3 * H]).astype(fc),
        w1bmc=np.ascontiguousarray(w1[H : 2 * H] - w1[2 * H : 3 * H]).astype(fc),
        w1d=np.ascontiguousarray(w1[3 * H :]).astype(fc),
        b1=np.asarray(b1, f), w2a=np.asarray(w2).astype(fc),
        b2=np.asarray(b2, f), w3=np.asarray(w3).astype(fc),
        ident=np.eye(128, dtype=f), identc=np.eye(128).astype(fc),
        negbig=np.full((1, 128), -BIG).astype(fc),
        onescol=np.ones((1, 128)).astype(fc),
    )
    m_all = np.asarray(mask)
    has0 = (m_all == 0).any(axis=1)
    ln_all = np.where(has0, np.argmax(m_all == 0, axis=1), T_).astype(np.int64)
    order = np.argsort(-ln_all, kind="stable")
    idx_cores = [order[c::ncores] for c in range(ncores)]
    tt = np.arange(T_)
    # shared window profile (core 0 holds the per-octet longest -> max)
    n_t = (ln_all[idx_cores[0]][:, None] > tt[None, :]).sum(axis=0)
    wins = np.minimum(BL_, np.maximum(((n_t + 3) // 4) * 4, 0)).astype(int)
    wins = np.maximum.accumulate(wins[::-1])[::-1]  # non-increasing
    item_eb = np.asarray(item_eb)
    item_his_eb = np.asarray(item_his_eb)
    item_his_eb_sum = np.asarray(item_his_eb_sum)
    in_maps = []
    for c in range(ncores):
        idx = idx_cores[c]
        ln = ln_all[idx]
        valid = tt[None, :] < ln[:, None]
        im = dict(shared)
        im["xT"] = np.ascontiguousarray(
            item_his_eb[idx].transpose(1, 2, 0)).astype(fc)
        im["qT"] = np.ascontiguousarray(item_eb[idx, 0].T).astype(fc)
        im["qN"] = np.ascontiguousarray(item_eb[idx, 0]).astype(f)
        im["hsum"] = np.ascontiguousarray(item_his_eb_sum[idx]).astype(f)
        im["validBT"] = valid.astype(np.uint8)
        im["negmBT"] = np.where(valid, 0.0, NEG).astype(f)
        im["deadT"] = np.ascontiguousarray((~valid).T).astype(fc)
        in_maps.append(im)
    perm = np.concatenate(idx_cores)
    return in_maps, perm, [int(v) for v in wins]


_prog_cache = {}


def kernel(**inputs):
    in_maps, perm, wins = host_prep(**inputs)
    key = tuple(wins)
    if key not in _prog_cache:
        _prog_cache[key] = build_program(windows=wins)
    nc = _prog_cache[key]
    res = run_bass_kernel_spmd(nc, in_maps, list(range(NCORES)))
    sorted_out = np.concatenate(
        [res.results[c]["out"] for c in range(NCORES)], axis=0)
    out = np.empty_like(sorted_out)
    out[perm] = sorted_out
    return out



# revision 32
# speedup vs baseline: 1.0280x; 1.0280x over previous
"""DIEN layer (GRU + attention + AUGRU) Trainium2 Bass kernel.

Pure data parallel across 8 NeuronCores: batch 2048 -> 256 per core.

Device layout: features on SBUF partitions, batch on the free dim; all
matmuls keep state in [feat, batch] layout so the recurrence never
transposes.  Ragged sequences: for t >= seq_len(b) the update gate is
saturated (v = 1-u -> 0) by adding -BIG to the (negated) u-gate
preactivation via a K=1 matmul, which freezes h exactly; the attention
softmax masks dead positions to exp(NEG-max) = 0 so alphas are exactly
0 there and the AUGRU also freezes.  Compute dtype for matmuls and
elementwise is bf16 (fp32 PSUM accumulation, fp32 softmax); set
dtc_name="f32" for a full-precision (4x slower matmul) variant.
"""

import sys

sys.path.insert(0, "/opt/trn_rl_repo")

import numpy as np
import ml_dtypes

import concourse.bacc as bacc
import concourse.mybir as mybir
import concourse.tile as tile
from concourse.bass_utils import run_bass_kernel_spmd

B, T, D, H = 2048, 200, 128, 128
NCORES = 8
BL = B // NCORES

BIG = 30000.0
NEG = np.float32(-(2.0**32) + 1.0)

F32 = mybir.dt.float32
BF16 = mybir.dt.bfloat16


def build_program(T_=T, BL_=BL, dtc_name="bf16", parts="all", windows=None):
    """Build the single-core program (run SPMD across 8 cores)."""
    nc = bacc.Bacc("TRN2", target_bir_lowering=False, debug=False)
    dt = F32
    dtc = BF16 if dtc_name == "bf16" else F32
    wins = list(windows) if windows is not None else [BL_] * T_
    assert len(wins) == T_ and all(0 <= w <= BL_ for w in wins)

    def dram(name, shape, dty=dt, kind="ExternalInput"):
        return nc.dram_tensor(name, shape, dty, kind=kind).ap()

    # ---- external inputs (per core); compute-dtype tensors use dtc ----
    xT = dram("xT", [T_, D, BL_], dtc)
    qT = dram("qT", [D, BL_], dtc)
    qN = dram("qN", [BL_, D])
    hsum = dram("hsum", [BL_, D])
    validBT = dram("validBT", [BL_, T_], mybir.dt.uint8)
    negmBT = dram("negmBT", [BL_, T_])
    deadT = dram("deadT", [T_, BL_], dtc)

    w1x = dram("w1x", [D, 2 * H], dtc)
    w1h = dram("w1h", [H, 2 * H], dtc)
    w1hn = dram("w1hn", [H, 2 * H], dtc)
    c1x = dram("c1x", [D, H], dtc)
    c1h = dram("c1h", [H, H], dtc)
    gb1n = dram("gb1n", [2 * H])
    gbw1r = dram("gbw1r", [1, 128], dtc)
    gbw1u = dram("gbw1u", [1, 128], dtc)
    gbw2r = dram("gbw2r", [1, 128], dtc)
    gbw2u = dram("gbw2u", [1, 128], dtc)
    cb1 = dram("cb1", [H])
    w2x = dram("w2x", [H, 2 * H], dtc)
    w2h = dram("w2h", [H, 2 * H], dtc)
    w2hn = dram("w2hn", [H, 2 * H], dtc)
    c2x = dram("c2x", [H, H], dtc)
    c2h = dram("c2h", [H, H], dtc)
    gb2n = dram("gb2n", [2 * H])
    cb2 = dram("cb2", [H])

    wq = dram("wq", [D, H], dtc)
    bq = dram("bq", [H])
    pra = dram("pra", [H])
    pra1m = dram("pra1m", [H])
    w1apc = dram("w1apc", [H, 80], dtc)
    w1bmc = dram("w1bmc", [H, 80], dtc)
    w1d = dram("w1d", [H, 80], dtc)
    b1 = dram("b1", [80])
    w2a = dram("w2a", [80, 40], dtc)
    b2 = dram("b2", [40])
    w3 = dram("w3", [40, 1], dtc)
    ident = dram("ident", [128, 128])
    identc = dram("identc", [128, 128], dtc)
    negbig = dram("negbig", [1, 128], dtc)
    onescol = dram("onescol", [1, 128], dtc)

    out = dram("out", [BL_, 3 * D + H], dt, kind="ExternalOutput")

    # DRAM scratch: (1 - alpha) rows, compute dtype
    aTd = nc.dram_tensor("aTd", [T_, BL_], dtc).ap()

    DCH = 8  # steps of deadrow/alpha rows per [1, DCH*BL] chunk

    SIG = mybir.ActivationFunctionType.Sigmoid
    TANH = mybir.ActivationFunctionType.Tanh
    EXP = mybir.ActivationFunctionType.Exp
    RELU = mybir.ActivationFunctionType.Relu
    COPYF = mybir.ActivationFunctionType.Copy
    AX = mybir.AxisListType.X
    MUL = mybir.AluOpType.mult
    SUB = mybir.AluOpType.subtract
    ADDOP = mybir.AluOpType.add
    MAXOP = mybir.AluOpType.max

    with tile.TileContext(nc) as tc:
        with tc.tile_pool(name="wts", bufs=1) as wp:

            def load_w(ap, shape, tag, col=False, dty=dtc):
                t_ = wp.tile(shape, dty, tag=tag, name=tag)
                if col:
                    n = ap.shape[0]
                    if n <= 128:
                        nc.sync.dma_start(t_[:, 0:1], ap.rearrange("(h a) -> h a", a=1))
                    else:
                        nc.sync.dma_start(t_[:], ap.rearrange("(a h) -> h a", h=128))
                else:
                    nc.sync.dma_start(t_[:], ap)
                return t_

            W1x = load_w(w1x, [D, 2 * H], "W1x")
            W1h = load_w(w1h, [H, 2 * H], "W1h")
            W1hn = load_w(w1hn, [H, 2 * H], "W1hn")
            C1x = load_w(c1x, [D, H], "C1x")
            C1h = load_w(c1h, [H, H], "C1h")
            W2x = load_w(w2x, [H, 2 * H], "W2x")
            W2h = load_w(w2h, [H, 2 * H], "W2h")
            W2hn = load_w(w2hn, [H, 2 * H], "W2hn")
            C2x = load_w(c2x, [H, H], "C2x")
            C2h = load_w(c2h, [H, H], "C2h")
            GB1 = load_w(gb1n, [128, 2], "GB1", col=True, dty=dt)
            GBW1 = (load_w(gbw1r, [1, 128], "GBW1r"),
                    load_w(gbw1u, [1, 128], "GBW1u"))
            GBW2 = (load_w(gbw2r, [1, 128], "GBW2r"),
                    load_w(gbw2u, [1, 128], "GBW2u"))
            CB1 = load_w(cb1, [H, 1], "CB1", col=True, dty=dt)
            GB2 = load_w(gb2n, [128, 2], "GB2", col=True, dty=dt)
            CB2 = load_w(cb2, [H, 1], "CB2", col=True, dty=dt)
            WQ = load_w(wq, [D, H], "WQ")
            BQ = load_w(bq, [H, 1], "BQ", col=True, dty=dt)
            PRA = load_w(pra, [H, 1], "PRA", col=True, dty=dt)
            PRA1M = load_w(pra1m, [H, 1], "PRA1M", col=True, dty=dt)
            W1APC = load_w(w1apc, [H, 80], "W1APC")
            W1BMC = load_w(w1bmc, [H, 80], "W1BMC")
            W1D = load_w(w1d, [H, 80], "W1D")
            B1 = load_w(b1, [80, 1], "B1", col=True, dty=dt)
            W2A = load_w(w2a, [80, 40], "W2A")
            B2 = load_w(b2, [40, 1], "B2", col=True, dty=dt)
            W3 = load_w(w3, [40, 1], "W3")
            IDN = load_w(ident, [128, 128], "IDN", dty=dt)
            IDNC = load_w(identc, [128, 128], "IDNC")
            NBIG = load_w(negbig, [1, 128], "NBIG")
            ONEC = load_w(onescol, [1, 128], "ONEC")

            QT = wp.tile([D, BL_], dtc, tag="QT", name="QT")
            nc.sync.dma_start(QT[:], qT)
            # rnn1 lives in SBUF: [H, T*BL], one BL-wide slab per step
            RNN1 = wp.tile([H, T_ * BL_], dtc, tag="RNN1", name="RNN1")
            HZERO = wp.tile([H, BL_], dtc, tag="HZERO", name="HZERO")
            nc.vector.memset(HZERO[:], 0.0)
            nbh = (BL_ + 127) // 128
            VAL, NEGM = [], []
            for i in range(nbh):
                p = min(128, BL_ - i * 128)
                v_ = wp.tile([128, T_], mybir.dt.uint8, tag=f"VAL{i}", name=f"VAL{i}")
                nc.sync.dma_start(v_[0:p, :], validBT[i * 128 : i * 128 + p, :])
                VAL.append(v_)
                n_ = wp.tile([128, T_], dt, tag=f"NEGM{i}", name=f"NEGM{i}")
                nc.sync.dma_start(n_[0:p, :], negmBT[i * 128 : i * 128 + p, :])
                NEGM.append(n_)

            # ---- attention query path (once) ----
            with (
                tc.tile_pool(name="ps_small", bufs=2, space="PSUM") as psp,
                tc.tile_pool(name="setup_tmp", bufs=2) as stp,
            ):
                p_qp = psp.tile([H, BL_], dt, tag="p_qp")
                nc.tensor.matmul(p_qp[:], WQ[:], QT[:], start=True, stop=True)
                qpre = stp.tile([H, BL_], dt, tag="qpre")
                nc.scalar.add(qpre[:], p_qp[:], BQ[:, 0:1])
                r1 = stp.tile([H, BL_], dt, tag="r1")
                nc.scalar.activation(r1[:], qpre[:], RELU, scale=PRA1M[:, 0:1])
                QP = wp.tile([H, BL_], dtc, tag="QP", name="QP")
                nc.vector.scalar_tensor_tensor(
                    QP[:], qpre[:], PRA[:, 0:1], r1[:], op0=MUL, op1=ADDOP
                )
                p_qc = psp.tile([80, BL_], dt, tag="p_qc")
                nc.tensor.matmul(p_qc[:], W1APC[:], QP[:], start=True, stop=True)
                QC = wp.tile([80, BL_], dt, tag="QC", name="QC")
                nc.scalar.add(QC[:], p_qc[:], B1[:, 0:1])

            # =================== GRU recurrences ===================
            def gru_pass(W_x, W_h, W_hn, C_x, C_h, GBn, CBc, x_of_t, store_rnn1,
                         use_alpha, hout=None, post_step=None, post_g0=None):
                # store_rnn1: write h into RNN1 SBUF regions (no h tiles)
                # Recurrent gate matmuls consume (vc, p_) of the previous step
                # (h' = vc - p_) so the h'-subtract runs off the critical path.
                with (
                    tc.tile_pool(name="g_x", bufs=4) as xp,
                    tc.tile_pool(name="g_h", bufs=3) as hp,
                    tc.tile_pool(name="g_rv", bufs=3) as rvp,
                    tc.tile_pool(name="g_tmp", bufs=6) as tp,
                    tc.tile_pool(name="g_dead", bufs=3) as dp,
                    tc.tile_pool(name="g_pg", bufs=2, space="PSUM") as pgp,
                    tc.tile_pool(name="g_pc", bufs=1, space="PSUM") as pcp,
                    tc.tile_pool(name="g_pa", bufs=1, space="PSUM") as pap,
                ):
                    NG = 2 if BL_ % 2 == 0 else 1  # pipeline groups (batch halves)
                    GW = BL_ // NG
                    inplace_h = use_alpha  # AUGRU: frozen suffix is the output
                    hs_ = []
                    for g in range(NG):
                        if store_rnn1:
                            hs_.append(HZERO[:, g * GW : (g + 1) * GW])
                        else:
                            h_g = hp.tile([H, GW], dtc, tag=f"h{g}", name=f"h{g}")
                            nc.vector.memset(h_g[:], 0.0)
                            hs_.append(h_g)
                    dead_ch = None
                    alpha_ch = None
                    prev_vc = [None] * NG
                    prev_p = [None] * NG
                    for t in range(T_):
                        j = t % DCH
                        if j == 0:
                            n_in = min(DCH, T_ - t)
                            dead_ch = dp.tile([1, DCH * BL_], dtc, tag="dead")
                            nc.sync.dma_start(
                                dead_ch[0:1, 0 : n_in * BL_],
                                deadT[t : t + n_in, :].rearrange(
                                    "(c a) b -> c (a b)", c=1),
                            )
                            if use_alpha:
                                alpha_ch = dp.tile([1, DCH * BL_], dtc, tag="alpha")
                                nc.sync.dma_start(
                                    alpha_ch[0:1, 0 : n_in * BL_],
                                    aTd[t : t + n_in, :].rearrange(
                                        "(c a) b -> c (a b)", c=1),
                                )
                        if wins[t] == 0:
                            continue
                        x_t = x_of_t(xp, t)
                        acts = []
                        for g in range(NG):
                            wg = min(max(wins[t] - g * GW, 0), GW)
                            if wg > 0:
                                acts.append((g, wg, g * GW))
                        # Group-major emission; recurrent matmuls consume
                        # (vc, p_) of the previous step so the h'-subtract is
                        # off the matmul critical path.
                        for g, wg, lo in acts:
                            xg = x_t[:, lo : lo + wg]
                            hg = hs_[g][:, 0:wg]
                            pv, pp = prev_vc[g], prev_p[g]
                            pg = pgp.tile([128, 2 * GW], dt, tag=f"pg{g}",
                                          name=f"pg{g}")
                            nc.tensor.matmul(pg[:, 0:wg], W_x[:, 0:H], xg,
                                             start=True, stop=(pv is None))
                            if pv is not None:
                                nc.tensor.matmul(pg[:, 0:wg], W_hn[:, 0:H],
                                                 pp[:, 0:wg],
                                                 start=False, stop=False)
                                nc.tensor.matmul(pg[:, 0:wg], W_h[:, 0:H],
                                                 pv[:, 0:wg],
                                                 start=False, stop=True)
                            nc.tensor.matmul(pg[:, GW : GW + wg], W_x[:, H:], xg,
                                             start=True, stop=False)
                            nc.tensor.matmul(
                                pg[:, GW : GW + wg], NBIG[:],
                                dead_ch[0:1, j * BL_ + lo : j * BL_ + lo + wg],
                                start=False, stop=(pv is None))
                            if pv is not None:
                                nc.tensor.matmul(pg[:, GW : GW + wg],
                                                 W_hn[:, H:], pp[:, 0:wg],
                                                 start=False, stop=False)
                                nc.tensor.matmul(pg[:, GW : GW + wg],
                                                 W_h[:, H:], pv[:, 0:wg],
                                                 start=False, stop=True)
                            rv = rvp.tile([128, 2 * GW], dtc, tag=f"rv{g}",
                                          name=f"rv{g}")
                            nc.scalar.activation(rv[:, 0:wg], pg[:, 0:wg], SIG,
                                                 bias=GBn[:, 0:1])
                            nc.scalar.activation(rv[:, GW : GW + wg],
                                                 pg[:, GW : GW + wg], SIG,
                                                 bias=GBn[:, 1:2])
                            rh = tp.tile([H, GW], dtc, tag=f"rh{g}",
                                         name=f"rh{g}")
                            nc.vector.tensor_mul(rh[:, 0:wg], rv[:, 0:wg], hg)
                            pc = pcp.tile([H, GW], dt, tag=f"pc{g}",
                                          name=f"pc{g}")
                            nc.tensor.matmul(pc[:, 0:wg], C_x[:], xg,
                                             start=True, stop=False)
                            nc.tensor.matmul(pc[:, 0:wg], C_h[:], rh[:, 0:wg],
                                             start=False, stop=True)
                            c = tp.tile([H, GW], dtc, tag=f"c{g}", name=f"c{g}")
                            nc.scalar.activation(c[:, 0:wg], pc[:, 0:wg], TANH,
                                                 bias=CBc[:, 0:1])
                            if use_alpha:
                                pa = pap.tile([128, GW], dt, tag=f"pa{g}",
                                              name=f"pa{g}")
                                nc.tensor.matmul(
                                    pa[:, 0:wg], ONEC[:],
                                    alpha_ch[0:1, j * BL_ + lo : j * BL_ + lo + wg],
                                    start=True, stop=True)
                                u1 = tp.tile([H, GW], dtc, tag=f"u1{g}",
                                             name=f"u1{g}")
                                nc.vector.scalar_tensor_tensor(
                                    u1[:, 0:wg], rv[:, GW : GW + wg], 1.0,
                                    pa[:, 0:wg], op0=SUB, op1=MUL)
                                p_ = tp.tile([H, GW], dtc, tag=f"p_{g}",
                                             name=f"p_{g}")
                                nc.vector.tensor_mul(p_[:, 0:wg], u1[:, 0:wg], hg)
                                vc = tp.tile([H, GW], dtc, tag=f"vc{g}",
                                             name=f"vc{g}")
                                nc.vector.scalar_tensor_tensor(
                                    vc[:, 0:wg], u1[:, 0:wg], 1.0, c[:, 0:wg],
                                    op0=ADDOP, op1=MUL)
                            else:
                                p_ = tp.tile([H, GW], dtc, tag=f"p_{g}",
                                             name=f"p_{g}")
                                nc.vector.scalar_tensor_tensor(
                                    p_[:, 0:wg], rv[:, GW : GW + wg], 1.0, hg,
                                    op0=SUB, op1=MUL)
                                vc = tp.tile([H, GW], dtc, tag=f"vc{g}",
                                             name=f"vc{g}")
                                nc.vector.tensor_mul(vc[:, 0:wg],
                                                     rv[:, GW : GW + wg],
                                                     c[:, 0:wg])
                            if inplace_h:
                                nc.vector.tensor_sub(hs_[g][:, 0:wg],
                                                     vc[:, 0:wg], p_[:, 0:wg])
                            elif store_rnn1:
                                reg = RNN1[:, t * BL_ + lo : t * BL_ + lo + GW]
                                nc.vector.tensor_sub(reg[:, 0:wg], vc[:, 0:wg],
                                                     p_[:, 0:wg])
                                hs_[g] = reg
                            else:
                                h2 = hp.tile([H, GW], dtc, tag=f"h{g}",
                                             name=f"h{g}")
                                nc.vector.tensor_sub(h2[:, 0:wg], vc[:, 0:wg],
                                                     p_[:, 0:wg])
                                hs_[g] = h2
                            prev_vc[g], prev_p[g] = vc, p_
                            if post_g0 is not None and g == 0:
                                post_g0(t)
                        if post_step is not None:
                            post_step(t, hs_, GW)
                    if hout is not None:
                        for g in range(NG):
                            nc.vector.tensor_copy(
                                hout[:, g * GW : (g + 1) * GW], hs_[g][:])

            XB = 4
            xT_p = xT.rearrange("t p b -> p t b")
            xcache = {}

            def x_from_xT(xp, t):
                t0 = (t // XB) * XB
                if t0 not in xcache:
                    nb = min(XB, T_ - t0)
                    xc = xp.tile([D, XB * BL_], dtc, tag="x", name=f"x{t0}")
                    nc.sync.dma_start(
                        xc[:].rearrange("p (a b) -> p a b", a=XB)[:, 0:nb, :],
                        xT_p[:, t0 : t0 + nb, :])
                    xcache.clear()
                    xcache[t0] = xc
                return xcache[t0][:, (t - t0) * BL_ : (t - t0 + 1) * BL_]

            hfin = wp.tile([H, BL_], dtc, tag="hfin", name="hfin")
            nc.vector.memset(hfin[:], 0.0)

            # ============ GRU1 with interleaved per-step attention ============
            do_attn = "attn" in parts or parts == "all"
            with (
                tc.tile_pool(name="a_tmp", bufs=4) as atp,
                tc.tile_pool(name="a_ps", bufs=2, space="PSUM") as app,
            ):
                # packed psum bank per step: s1 [0:BL], s2 [BL:2BL]; the two
                # score columns reuse s1's cols [0:2] after a1 consumed them
                SC0, SC1, SC2 = 0, BL_, 0
                scores_sb = wp.tile([128, 2 * T_], dt, tag="scores",
                                    name="scores")
                nc.vector.memset(scores_sb[:], 0.0)
                sc_v = scores_sb[:].rearrange("p (i t) -> p i t", i=2)

                # Attention score FCN processed in PAIRS of GRU steps,
                # lag-2 pipelined into the ACT-idle slots of the recurrence:
                # psA MMs accumulate per step; sigma1 for a completed pair
                # fires at the next step's post_g0 slot; W2A/sigma2/W3/score
                # copies at the next post_step. Column layout per pair bank:
                # a1pre even step at [0:80, 0:BL], odd at [0:80, BL:2BL];
                # a2pre overwrites [0:40, ...] after sigma1; W3 scores land in
                # cols [0:4] as [i0e, i0o, i1e, i1o]. Stale-PSUM columns of
                # the combined ranges are finite garbage, masked in softmax.
                cur = {}
                rdy = {}
                fin = {}

                def attn_sig1(t):
                    if not do_attn or not rdy:
                        return
                    psA = rdy["psA"]
                    wtot = rdy["w0"] + rdy["w1"]
                    a1s = atp.tile([80, 2 * BL_], dtc, tag="a1s", name="a1s")
                    nc.scalar.activation(a1s[:, 0:wtot], psA[0:80, 0:wtot],
                                         SIG, bias=B1[:, 0:1])
                    rdy["a1s"] = a1s
                    fin.update(rdy)
                    rdy.clear()

                def attn_fin():
                    psA, a1s = fin["psA"], fin["a1s"]
                    t0, w0, t1, w1 = fin["t0"], fin["w0"], fin["t1"], fin["w1"]
                    wtot = w0 + w1
                    nc.tensor.matmul(psA[0:40, 0:wtot], W2A[:], a1s[:, 0:wtot],
                                     start=True, stop=True)
                    a2 = atp.tile([40, 2 * BL_], dtc, tag="a2", name="a2")
                    nc.scalar.activation(a2[:, 0:wtot], psA[0:40, 0:wtot],
                                         SIG, bias=B2[:, 0:1])
                    for i in range(nbh):
                        p = min(128, w0 - i * 128)
                        if p <= 0:
                            continue
                        p1 = max(0, min(128, w1 - i * 128))
                        nc.tensor.matmul(
                            psA[0:p, 2 * i : 2 * i + 1],
                            a2[:, i * 128 : i * 128 + p], W3[:],
                            start=True, stop=True)
                        if p1 > 0:
                            nc.tensor.matmul(
                                psA[0:p1, 2 * i + 1 : 2 * i + 2],
                                a2[:, w0 + i * 128 : w0 + i * 128 + p1],
                                W3[:], start=True, stop=True)
                            nc.vector.tensor_copy(
                                sc_v[:, i, :][0:p, t0 : t0 + 2],
                                psA[0:p, 2 * i : 2 * i + 2])
                        else:
                            nc.vector.tensor_copy(
                                sc_v[:, i, :][0:p, t0 : t0 + 1],
                                psA[0:p, 2 * i : 2 * i + 1])
                    fin.clear()

                def attn_step(t, hs_, GW):
                    if not do_attn:
                        return
                    if fin:
                        attn_fin()
                    elif rdy:
                        attn_sig1(t)
                    if t is None:
                        return
                    w = wins[t]
                    if not cur:
                        psA = app.tile([128, 2 * BL_], dt, tag="psA",
                                       name="psA")
                        cur.update(t0=t, w0=w, psA=psA, off=0)
                        off = 0
                    else:
                        psA = cur["psA"]
                        off = cur["w0"]
                    for g in range(len(hs_)):
                        wg = min(max(w - g * GW, 0), GW)
                        if wg == 0:
                            continue
                        lo = g * GW
                        hg = hs_[g][:, 0:wg]
                        prod = atp.tile([H, GW], dtc, tag=f"aprod{g}",
                                        name=f"aprod{g}")
                        nc.vector.tensor_mul(prod[:, 0:wg], hg,
                                             QP[:, lo : lo + wg])
                        dst = psA[0:80, off + lo : off + lo + wg]
                        nc.tensor.matmul(dst, W1BMC[:], hg,
                                         start=True, stop=False)
                        nc.tensor.matmul(dst, W1APC[:], QP[:, lo : lo + wg],
                                         start=False, stop=False)
                        nc.tensor.matmul(dst, W1D[:], prod[:, 0:wg],
                                         start=False, stop=True)
                    if off > 0:
                        cur.update(t1=t, w1=w)
                        rdy.update(cur)
                        cur.clear()

                use_attn = "gru1" in parts or parts == "all"
                gru_pass(W1x, W1h, W1hn, C1x, C1h, GB1, CB1, x_from_xT,
                         True, False,
                         post_step=attn_step if use_attn else None)
                if use_attn and do_attn:
                    if fin:
                        attn_fin()
                    if rdy:
                        attn_sig1(None)
                        attn_fin()

                # masked softmax; store abar = (1 - alpha) transposed to aTd
                nbh_sm = nbh if do_attn else 0
                with (
                    tc.tile_pool(name="a_sm", bufs=1) as smp,
                    tc.tile_pool(name="a_tr", bufs=2, space="PSUM") as trp,
                ):
                    for i in range(nbh_sm):
                        p = min(128, BL_ - i * 128)
                        sm = smp.tile([128, T_], dt, tag=f"sm{i}", name=f"sm{i}")
                        nc.vector.select(sm[0:p, :], VAL[i][0:p, :],
                                         sc_v[:, i, :][0:p, :], NEGM[i][0:p, :])
                        nmx = smp.tile([128, 1], dt, tag=f"nmx{i}", name=f"nmx{i}")
                        nc.vector.tensor_reduce(
                            nmx[0:p, :], sm[0:p, :], axis=AX, op=MAXOP, negate=True)
                        ex = smp.tile([128, T_], dt, tag=f"ex{i}", name=f"ex{i}")
                        nc.scalar.activation(ex[0:p, :], sm[0:p, :], EXP,
                                             bias=nmx[0:p, 0:1])
                        sume = smp.tile([128, 1], dt, tag=f"sume{i}", name=f"sume{i}")
                        nc.vector.tensor_reduce(
                            sume[0:p, :], ex[0:p, :], axis=AX, op=ADDOP)
                        rec = smp.tile([128, 1], dt, tag=f"rec{i}", name=f"rec{i}")
                        nc.vector.reciprocal(rec[0:p, :], sume[0:p, :])
                        alp = smp.tile([128, T_], dt, tag=f"alp{i}", name=f"alp{i}")
                        nc.vector.tensor_scalar_mul(alp[0:p, :], ex[0:p, :],
                                                    rec[0:p, 0:1])
                        for c0 in range(0, T_, 128):
                            w2_ = min(128, T_ - c0)
                            pt = trp.tile([128, 128], dt, tag="p_tr")
                            nc.tensor.transpose(
                                pt[0:w2_, 0:p], alp[0:p, c0 : c0 + w2_],
                                IDN[0:p, 0:p])
                            st = smp.tile([128, 128], dtc, tag="st", name="st")
                            # abar = 1 - alpha, cast to compute dtype
                            nc.scalar.activation(
                                st[0:w2_, 0:p], pt[0:w2_, 0:p], COPYF,
                                bias=1.0, scale=-1.0)
                            nc.sync.dma_start(
                                aTd[c0 : c0 + w2_, i * 128 : i * 128 + p],
                                st[0:w2_, 0:p])

            # =================== AUGRU ===================
            def x_from_rnn1(xp, t):
                return RNN1[:, t * BL_ : (t + 1) * BL_]

            if "augru" in parts or parts == "all":
                gru_pass(W2x, W2h, W2hn, C2x, C2h, GB2, CB2, x_from_rnn1,
                         False, True, hout=hfin)

            # =================== output assembly ===================
            with (
                tc.tile_pool(name="o_t", bufs=2) as otp,
                tc.tile_pool(name="o_p", bufs=2, space="PSUM") as opp,
            ):
                for i in range(nbh if parts == "all" else 0):
                    p = min(128, BL_ - i * 128)
                    qs = otp.tile([128, D], dt, tag="qs")
                    nc.sync.dma_start(qs[0:p, :], qN[i * 128 : i * 128 + p, :])
                    hs = otp.tile([128, D], dt, tag="hs")
                    nc.sync.dma_start(hs[0:p, :], hsum[i * 128 : i * 128 + p, :])
                    pr = otp.tile([128, D], dt, tag="pr")
                    nc.vector.tensor_mul(pr[0:p, :], qs[0:p, :], hs[0:p, :])
                    nc.sync.dma_start(out[i * 128 : i * 128 + p, 0:D], qs[0:p, :])
                    nc.sync.dma_start(out[i * 128 : i * 128 + p, D : 2 * D], hs[0:p, :])
                    nc.sync.dma_start(out[i * 128 : i * 128 + p, 2 * D : 3 * D],
                                      pr[0:p, :])
                    ptr = opp.tile([128, 128], dtc, tag="ptr")
                    nc.tensor.transpose(ptr[0:p, :], hfin[:, i * 128 : i * 128 + p],
                                        IDNC[:])
                    ht = otp.tile([128, H], dt, tag="ht")
                    nc.scalar.copy(ht[0:p, :], ptr[0:p, :])
                    nc.sync.dma_start(out[i * 128 : i * 128 + p, 3 * D :], ht[0:p, :])

    nc.compile()
    return nc


def host_prep(item_eb, item_his_eb, item_his_eb_sum, mask,
              gk1, gb1, ck1, cb1,
              wq, bq, prelu_alpha, w1, b1, w2, b2, w3, b3,
              gk2, gb2, ck2, cb2, T_=T, BL_=BL, ncores=NCORES, dtc_name="bf16"):
    """Shard + preprocess. Samples are sorted by seq_len (descending) and
    dealt round-robin to cores so every core shares one length profile;
    returns (in_maps, perm, windows) where windows[t] is the 32-rounded
    max active-batch width at step t (same on every core by construction),
    and out[perm] = concat(core outputs) restores the original order."""
    f = np.float32
    fc = ml_dtypes.bfloat16 if dtc_name == "bf16" else np.float32

    w1x = np.ascontiguousarray(gk1[:D]).astype(f)
    w1h = np.ascontiguousarray(gk1[D:]).astype(f)
    w1x[:, H:] = -w1x[:, H:]
    w1h[:, H:] = -w1h[:, H:]
    gb1n = np.asarray(gb1, f).copy()
    gb1n[H:] = -gb1n[H:]
    w2x_ = np.ascontiguousarray(gk2[:H]).astype(f)
    w2h_ = np.ascontiguousarray(gk2[H:]).astype(f)
    w2x_[:, H:] = -w2x_[:, H:]
    w2h_[:, H:] = -w2h_[:, H:]
    gb2n = np.asarray(gb2, f).copy()
    gb2n[H:] = -gb2n[H:]
    shared = dict(
        w1x=w1x.astype(fc), w1h=w1h.astype(fc), w1hn=(-w1h).astype(fc),
        w2hn=(-w2h_).astype(fc),
        c1x=np.ascontiguousarray(ck1[:D]).astype(fc),
        c1h=np.ascontiguousarray(ck1[D:]).astype(fc),
        gb1n=gb1n, cb1=np.asarray(cb1, f),
        w2x=w2x_.astype(fc), w2h=w2h_.astype(fc),
        c2x=np.ascontiguousarray(ck2[:H]).astype(fc),
        c2h=np.ascontiguousarray(ck2[H:]).astype(fc),
        gb2n=gb2n, cb2=np.asarray(cb2, f),
        gbw1r=gb1n[None, :H].astype(fc), gbw1u=gb1n[None, H:].astype(fc),
        gbw2r=gb2n[None, :H].astype(fc), gbw2u=gb2n[None, H:].astype(fc),
        wq=np.asarray(wq).astype(fc), bq=np.asarray(bq, f),
        pra=np.asarray(prelu_alpha, f),
        pra1m=(1.0 - np.asarray(prelu_alpha, f)),
        w1apc=np.ascontiguousarray(w1[:H] + w1[2 * H : 3 * H]).astype(fc),
        w1bmc=np.ascontiguousarray(w1[H : 2 * H] - w1[2 * H : 3 * H]).astype(fc),
        w1d=np.ascontiguousarray(w1[3 * H :]).astype(fc),
        b1=np.asarray(b1, f), w2a=np.asarray(w2).astype(fc),
        b2=np.asarray(b2, f), w3=np.asarray(w3).astype(fc),
        ident=np.eye(128, dtype=f), identc=np.eye(128).astype(fc),
        negbig=np.full((1, 128), -BIG).astype(fc),
        onescol=np.ones((1, 128)).astype(fc),
    )
    m_all = np.asarray(mask)
    has0 = (m_all == 0).any(axis=1)
    ln_all = np.where(has0, np.argmax(m_all == 0, axis=1), T_).astype(np.int64)
    order = np.argsort(-ln_all, kind="stable")
    idx_cores = [order[c::ncores] for c in range(ncores)]
    tt = np.arange(T_)
    # shared window profile (core 0 holds the per-octet longest -> max)
    n_t = (ln_all[idx_cores[0]][:, None] > tt[None, :]).sum(axis=0)
    wins = np.minimum(BL_, np.maximum(((n_t + 3) // 4) * 4, 0)).astype(int)
    wins = np.maximum.accumulate(wins[::-1])[::-1]  # non-increasing
    item_eb = np.asarray(item_eb)
    item_his_eb = np.asarray(item_his_eb)
    item_his_eb_sum = np.asarray(item_his_eb_sum)
    in_maps = []
    for c in range(ncores):
        idx = idx_cores[c]
        ln = ln_all[idx]
        valid = tt[None, :] < ln[:, None]
        im = dict(shared)
        im["xT"] = np.ascontiguousarray(
            item_his_eb[idx].transpose(1, 2, 0)).astype(fc)
        im["qT"] = np.ascontiguousarray(item_eb[idx, 0].T).astype(fc)
        im["qN"] = np.ascontiguousarray(item_eb[idx, 0]).astype(f)
        im["hsum"] = np.ascontiguousarray(item_his_eb_sum[idx]).astype(f)
        im["validBT"] = valid.astype(np.uint8)
        im["negmBT"] = np.where(valid, 0.0, NEG).astype(f)
        im["deadT"] = np.ascontiguousarray((~valid).T).astype(fc)
        in_maps.append(im)
    perm = np.concatenate(idx_cores)
    return in_maps, perm, [int(v) for v in wins]


_prog_cache = {}


def kernel(**inputs):
    in_maps, perm, wins = host_prep(**inputs)
    key = tuple(wins)
    if key not in _prog_cache:
        _prog_cache[key] = build_program(windows=wins)
    nc = _prog_cache[key]
    res = run_bass_kernel_spmd(nc, in_maps, list(range(NCORES)))
    sorted_out = np.concatenate(
        [res.results[c]["out"] for c in range(NCORES)], axis=0)
    out = np.empty_like(sorted_out)
    out[perm] = sorted_out
    return out

